# revision 1
# baseline (speedup 1.0000x reference)
"""Multi-head attention (B=4, S=2048, D=1024, H=16, hd=64) on 8 Trainium2
NeuronCores, tensor-parallel across heads (2 heads per core).

Strategy per core (head-pair p, heads 2p and 2p+1):
 - Host pre-transposes x to xT [D, B*S] bf16 (shared by all cores) and slices
   per-head-pair weight columns / proj rows.
 - QKV: weights stationary, xT moving -> qT/kT/vT layouts [128(2hx64hd), S]
   per batch, biases folded per-partition during PSUM eviction.
 - v is PE-transposed to natural [tok, hd] layout with an appended ones
   column; the attn@v matmul (lhsT=[v|1], M=65) then accumulates both the
   attention output AND the softmax denominators (PSUM row 64) for free.
 - Scores are computed transposed (sT = k q^T, contraction over hd=64, two
   heads packed into the two 64-row groups of the PE array, concurrent);
   softmax needs no max-subtraction (|s| <= ~3 by construction); exp on
   ScalarE straight from PSUM, bf16 out.
 - Normalization: denominator rows collected at legal SBUF base partitions
   {0,32,64,96}, one DVE reciprocal per 4 rows, GPSIMD partition-broadcast,
   one DVE multiply (in-place on the bf16 oT tile).
 - Proj: w_proj row-slice stationary, oT moving -> per-core partial yT
   [1024, B*S] fp32, DMA'd straight from PSUM to DRAM. Host sums the 8
   partials, transposes back and adds b_proj.

Emission is software-pipelined: batch b's attention (ScalarE-bound exp
stream) is interleaved with batch b+1's QKV/v-transpose and batch b-1's
normalize+proj so that every engine keeps work in its in-order queue.
"""
from contextlib import ExitStack
from itertools import chain, islice


def _take(gen, n):
    return islice(gen, n)

import numpy as np
import ml_dtypes

import concourse.mybir as mybir
import concourse.tile as tile
from concourse import bacc
from concourse.bass_utils import run_bass_kernel_spmd
from concourse.masks import make_identity

BF16 = mybir.dt.bfloat16
F32 = mybir.dt.float32

B, S, D, H = 4, 2048, 1024, 16
HD = D // H          # 64
T = B * S            # 8192 tokens
NB = D // 128        # 8 d-tiles
SQC = 512            # query-chunk
NSQ = S // SQC       # 4 chunks per batch
NSK = S // 128       # 16 key tiles per batch
EXP = mybir.ActivationFunctionType.Exp

_CACHE = {}


def _build(reps=1, ablate=()):
    nc = bacc.Bacc("TRN2", target_bir_lowering=False, debug=False, num_devices=8)
    xt_d = nc.dram_tensor("xt", [D, T], BF16, kind="ExternalInput").ap()
    wq_d = nc.dram_tensor("wq", [D, 128], BF16, kind="ExternalInput").ap()
    wk_d = nc.dram_tensor("wk", [D, 128], BF16, kind="ExternalInput").ap()
    wv_d = nc.dram_tensor("wv", [D, 128], BF16, kind="ExternalInput").ap()
    bq_d = nc.dram_tensor("bq", [128, 1], F32, kind="ExternalInput").ap()
    bk_d = nc.dram_tensor("bk", [128, 1], F32, kind="ExternalInput").ap()
    bv_d = nc.dram_tensor("bv", [128, 1], F32, kind="ExternalInput").ap()
    wp_d = nc.dram_tensor("wp", [128, D], BF16, kind="ExternalInput").ap()
    yt_d = nc.dram_tensor("yt", [D, T], BF16, kind="ExternalOutput").ap()

    with tile.TileContext(nc) as tc, ExitStack() as ctx:
        const = ctx.enter_context(tc.tile_pool(name="const", bufs=1))
        xtp = ctx.enter_context(tc.tile_pool(name="xt", bufs=2))
        qkvp = ctx.enter_context(tc.tile_pool(name="qkv", bufs=3))
        vsp = ctx.enter_context(tc.tile_pool(name="vs", bufs=2))
        ep = ctx.enter_context(tc.tile_pool(name="e", bufs=6))
        otp = ctx.enter_context(tc.tile_pool(name="ot", bufs=2))
        tp = ctx.enter_context(tc.tile_pool(name="t", bufs=2))
        rp = ctx.enter_context(tc.tile_pool(name="r", bufs=2))
        r0p = ctx.enter_context(tc.tile_pool(name="r0", bufs=4))
        rbp = ctx.enter_context(tc.tile_pool(name="rb", bufs=4))
        yp = ctx.enter_context(tc.tile_pool(name="y", bufs=4))
        # PSUM: "ps" = 1-bank scratch (scores double-buffered + qkv/proj/
        # transpose), "pso" = attention-output accumulators (1 bank per head)
        psp = ctx.enter_context(tc.tile_pool(name="ps", bufs=6, space="PSUM"))
        pso = ctx.enter_context(tc.tile_pool(name="pso", bufs=1, space="PSUM"))

        # persistent weights
        wq = const.tile([128, NB * 128], BF16)
        wk = const.tile([128, NB * 128], BF16)
        wv = const.tile([128, NB * 128], BF16)
        for w_sb, w_dr in ((wq, wq_d), (wk, wk_d), (wv, wv_d)):
            nc.sync.dma_start(
                w_sb[:].rearrange("p (n c) -> p n c", n=NB),
                w_dr.rearrange("(n p) c -> p n c", p=128))
        bq = const.tile([128, 1], F32)
        bk = const.tile([128, 1], F32)
        bv = const.tile([128, 1], F32)
        nc.sync.dma_start(bq[:], bq_d)
        nc.sync.dma_start(bk[:], bk_d)
        nc.sync.dma_start(bv[:], bv_d)
        wp = const.tile([128, D], BF16)
        nc.sync.dma_start(wp[:], wp_d)
        ident = const.tile([128, 128], BF16)
        make_identity(nc, ident[:])

        # per-batch state handed between pipeline stages
        st = {}

        def _qkv_chunk(b, w_sb, bias, dst, c):
            xt = st[b]["xt"]
            ps = psp.tile([128, SQC], F32, tag="ps")
            for d in range(NB):
                nc.tensor.matmul(
                    ps[:], w_sb[:, d * 128:(d + 1) * 128],
                    xt[:, d * S + c * SQC: d * S + (c + 1) * SQC],
                    start=(d == 0), stop=(d == NB - 1))
            nc.vector.tensor_scalar_add(
                dst[:, c * SQC:(c + 1) * SQC], ps[:], bias[:])

        def gen_pre_qk(b):
            """xt load (chunked) + q,k projections for batch b."""
            tok0 = b * S
            xt = xtp.tile([128, NB * S], BF16, tag="xt")
            st[b] = {"xt": xt}
            if b == 0:
                # chunk-major so the first token-chunk lands early
                for c in range(NSQ):
                    for d in range(NB):
                        nc.sync.dma_start(
                            xt[:, d * S + c * SQC: d * S + (c + 1) * SQC],
                            xt_d[d * 128:(d + 1) * 128,
                                 tok0 + c * SQC:tok0 + (c + 1) * SQC])
                    yield
            else:
                for d in range(NB):
                    nc.sync.dma_start(
                        xt[:, d * S:(d + 1) * S],
                        xt_d[d * 128:(d + 1) * 128, tok0:tok0 + S])
                    if d % 2:
                        yield
            qT = qkvp.tile([128, S], BF16, tag="qT")
            kT = qkvp.tile([128, S], BF16, tag="kT")
            st[b]["qT"] = qT
            st[b]["kT"] = kT
            for w_sb, bias, dst in ((wq, bq, qT), (wk, bk, kT)):
                for c in range(NSQ):
                    _qkv_chunk(b, w_sb, bias, dst, c)
                    yield

        def gen_pre_v(b):
            """v projection + transpose to natural layout for batch b."""
            vT = qkvp.tile([128, S], BF16, tag="vT")
            for c in range(NSQ):
                _qkv_chunk(b, wv, bv, vT, c)
                yield
            # v -> natural layout tiles [vA(64) | 1 | vB(64) | 1]
            v_sb = vsp.tile([128, NSK * 130], BF16, tag="vs")
            st[b]["v_sb"] = v_sb
            nc.vector.memset(v_sb[:], 1.0)
            yield
            for stk in range(NSK):
                ps_t = psp.tile([128, 128], BF16, tag="ps")
                nc.tensor.transpose(ps_t[:], vT[:, stk * 128:(stk + 1) * 128],
                                    ident[:])
                o0 = stk * 130
                nc.vector.tensor_copy(v_sb[:, o0:o0 + 64], ps_t[:, 0:64])
                nc.vector.tensor_copy(v_sb[:, o0 + 65:o0 + 129],
                                      ps_t[:, 64:128])
                yield

        def gen_attn(b):
            """flash attention for batch b (yield per (chunk, sk) step).
            tsb0 holds denominator rows of chunks 0-1, tsb1 of chunks 2-3,
            so normalization of the first half can start mid-batch."""
            qT, kT = st[b]["qT"], st[b]["kT"]
            oT = otp.tile([128, S], BF16, tag="ot")
            tsb0 = tp.tile([97, SQC], F32, tag="t0")
            tsb1 = tp.tile([97, SQC], F32, tag="t1")
            nc.vector.memset(tsb0[:], 1.0)
            nc.vector.memset(tsb1[:], 1.0)
            st[b]["oT"] = oT
            st[b]["tsb"] = (tsb0, tsb1)
            for c in range(NSQ):
                q0 = c * SQC
                v_sb = st[b]["v_sb"]
                o_A = pso.tile([65, SQC], F32, tag="oA")
                o_B = pso.tile([65, SQC], F32, tag="oB")
                for sk in range(NSK):
                    k0 = sk * 128
                    s_A = psp.tile([128, SQC], F32, tag="ps")
                    s_B = psp.tile([128, SQC], F32, tag="ps")
                    nc.tensor.matmul(s_A[:], kT[0:64, k0:k0 + 128],
                                     qT[0:64, q0:q0 + SQC],
                                     start=True, stop=True, tile_position=(0, 0))
                    nc.tensor.matmul(s_B[:], kT[64:128, k0:k0 + 128],
                                     qT[64:128, q0:q0 + SQC],
                                     start=True, stop=True, tile_position=(64, 0))
                    e_A = ep.tile([128, SQC], BF16, tag="e")
                    e_B = ep.tile([128, SQC], BF16, tag="e")
                    nc.scalar.activation(e_A[:], s_A[:], EXP, scale=0.125)
                    nc.scalar.activation(e_B[:], s_B[:], EXP, scale=0.125)
                    v0 = sk * 130
                    nc.tensor.matmul(o_A[:], v_sb[:, v0:v0 + 65], e_A[:],
                                     start=(sk == 0), stop=(sk == NSK - 1))
                    nc.tensor.matmul(o_B[:], v_sb[:, v0 + 65:v0 + 130], e_B[:],
                                     start=(sk == 0), stop=(sk == NSK - 1))
                    yield
                # stash unnormalized o (bf16) and the denominator row
                # (denominators of chunk pair c//2 go to tsb[c//2], at row
                # 32*(2*(c%2)+h))
                for h, o_ps in ((0, o_A), (1, o_B)):
                    row = (2 * (c % 2) + h) * 32
                    nc.vector.tensor_copy(
                        st[b]["tsb"][c // 2][row:row + 1, :], o_ps[64:65, :])
                    nc.vector.tensor_copy(oT[h * 64:(h + 1) * 64, q0:q0 + SQC],
                                          o_ps[0:64, :])
                yield

        def gen_post_half(b, half):
            """normalize + partial-proj + store for chunks 2h, 2h+1 of b."""
            tok0 = b * S
            oT = st[b]["oT"]
            tsb = st[b]["tsb"][half]
            rsb = rp.tile([97, SQC], F32, tag=f"r{half}t")
            if "recip" not in ablate:
                nc.vector.reciprocal(rsb[:], tsb[:])
            else:
                nc.vector.memset(rsb[:], 1.0)
            yield
            for c in (2 * half, 2 * half + 1):
                q0 = c * SQC
                for h in range(2):
                    row = (2 * (c % 2) + h) * 32
                    if row == 0:
                        r_ap = rsb[0:1, :]
                    else:
                        r0 = r0p.tile([1, SQC], F32, tag="r0")
                        nc.vector.tensor_copy(r0[:], rsb[row:row + 1, :])
                        r_ap = r0[:]
                    if "norm" not in ablate:
                        rb = rbp.tile([128, SQC], F32, tag="rb")
                        nc.gpsimd.partition_broadcast(rb[:], r_ap)
                        nc.vector.tensor_mul(
                            oT[h * 64:(h + 1) * 64, q0:q0 + SQC],
                            oT[h * 64:(h + 1) * 64, q0:q0 + SQC],
                            rb[h * 64:(h + 1) * 64, :])
                    yield
            # partial proj: yT[ct, tok] = wp[:, ct].T @ oT
            # both chunks of this half evicted into one y tile -> 1 DMA per
            # (col-tile, half)
            for ct in range(NB):
                y = yp.tile([128, 2 * SQC], BF16, tag="y")
                for ci, c in enumerate((2 * half, 2 * half + 1)):
                    ps = psp.tile([128, SQC], F32, tag="ps")
                    nc.tensor.matmul(ps[:], wp[:, ct * 128:(ct + 1) * 128],
                                     oT[:, c * SQC:(c + 1) * SQC],
                                     start=True, stop=True)
                    nc.vector.tensor_copy(
                        y[:, ci * SQC:(ci + 1) * SQC], ps[:])
                    yield
                nc.sync.dma_start(
                    yt_d[ct * 128:(ct + 1) * 128,
                         tok0 + 2 * half * SQC:tok0 + (2 * half + 2) * SQC],
                    y[:])

        def interleave(main, filler, n_main, n_fill):
            """emit main and filler streams at proportional rates."""
            ratio = max(n_fill, 1) / max(n_main, 1)
            credit = 0.0
            for mi in main:
                credit += ratio
                while credit >= 1.0:
                    credit -= 1.0
                    if next(filler, StopIteration) is StopIteration:
                        credit = -1e18
                        break
            for _ in filler:
                pass

        N_ATTN_HALF = 2 * (NSK + 1)
        N_PREQK = NSQ + 2 * NSQ
        N_PREV = NSQ + 2 + NSK
        N_POST = 1 + 4 + 2 * NB

        # Emission order IS dependency order for Tile, so a batch's qkv must
        # be fully emitted before its attention. Filler therefore runs one
        # batch ahead: during attn(b): pre_v(b+1) + post(b-1,1) in the first
        # half, post(b,0) + pre_qk(b+2) in the second.
        for _ in range(reps):
            for _ in gen_pre_qk(0):
                pass
            interleave(gen_pre_v(0), gen_pre_qk(1), N_PREV, N_PREQK)
            for b in range(B):
                at = gen_attn(b)
                f1, n1 = [], 0
                if b - 1 >= 0:
                    f1.append(gen_post_half(b - 1, 1))
                    n1 += N_POST
                if b + 1 < B:
                    f1.append(gen_pre_v(b + 1))
                    n1 += N_PREV
                interleave(_take(at, N_ATTN_HALF), chain(*f1), N_ATTN_HALF, n1)
                f2, n2 = [gen_post_half(b, 0)], N_POST
                if b + 2 < B:
                    f2.append(gen_pre_qk(b + 2))
                    n2 += N_PREQK
                interleave(at, chain(*f2), N_ATTN_HALF, n2)
            for _ in gen_post_half(B - 1, 1):
                pass
    nc.compile()
    return nc


def _get_nc(reps=1, ablate=()):
    key = f"nc{reps}{ablate}"
    if key not in _CACHE:
        _CACHE[key] = _build(reps, ablate)
    return _CACHE[key]


def make_in_maps(x, w_qkv, b_qkv, w_proj):
    """Host-side sharding: slice/cast per-core inputs."""
    bf16 = ml_dtypes.bfloat16
    xt = np.ascontiguousarray(
        np.asarray(x, dtype=np.float32).reshape(T, D).T).astype(bf16)
    w_qkv = np.asarray(w_qkv, dtype=np.float32)
    b_qkv = np.asarray(b_qkv, dtype=np.float32)
    w_proj = np.asarray(w_proj, dtype=np.float32)
    in_maps = []
    for p in range(8):
        c0 = p * 128          # first of the 128 head-pair columns
        in_maps.append({
            "xt": xt,
            "wq": np.ascontiguousarray(w_qkv[:, c0:c0 + 128]).astype(bf16),
            "wk": np.ascontiguousarray(w_qkv[:, D + c0:D + c0 + 128]).astype(bf16),
            "wv": np.ascontiguousarray(w_qkv[:, 2 * D + c0:2 * D + c0 + 128]).astype(bf16),
            "bq": b_qkv[c0:c0 + 128].reshape(128, 1).copy(),
            "bk": b_qkv[D + c0:D + c0 + 128].reshape(128, 1).copy(),
            "bv": b_qkv[2 * D + c0:2 * D + c0 + 128].reshape(128, 1).copy(),
            "wp": np.ascontiguousarray(w_proj[c0:c0 + 128, :]).astype(bf16),
        })
    return in_maps


def combine_outputs(results, b_proj):
    """Host-side unshard: sum partial yT, transpose back, add bias."""
    acc = np.zeros((D, T), np.float32)
    for r in results:
        acc += np.asarray(r["yt"], dtype=np.float32)
    y = acc.T.reshape(B, S, D) + np.asarray(b_proj, dtype=np.float32)
    return y.astype(np.float32)


def kernel(x, w_qkv, b_qkv, w_proj, b_proj):
    nc = _get_nc()
    in_maps = make_in_maps(x, w_qkv, b_qkv, w_proj)
    res = run_bass_kernel_spmd(nc, in_maps, list(range(8)))
    return combine_outputs(res.results, b_proj)



# revision 2
# speedup vs baseline: 1.2345x; 1.2345x over previous
"""Multi-head attention (B=4, S=2048, D=1024, H=16, hd=64) on 8 Trainium2
NeuronCores, tensor-parallel across heads (2 heads per core).

v2 changes over the baseline:
 - Scores for both heads land in ONE 2-bank PSUM tile [128, 1024]; softmax
   exp is a single N=1024 instruction per (chunk, key-tile) instead of two
   N=512 ones (halves ScalarE per-instruction overhead).
 - A fraction of the exp sites run on the (otherwise copy-bound) Vector
   engine via a custom Schraudolph exp op: bits = s*C0 + C1 written to a
   uint16 tile and bitcast to bf16 (max rel err ~3.4%, washes out in the
   softmax average); this splits the exp stream across two engines.
 - v natural-layout build: 4 PE transposes into one PSUM bank, ONE strided
   DVE copy into the [vA|1|vB|1] layout, ones written once by a strided
   memset (replaces 32 small copies + full-tile memset per batch).
 - QKV bias-add evictions moved to ScalarE (Identity shares the exp table
   set, so no table reloads); denominator row relocation copies moved to
   the idle GPSIMD engine.
"""
from contextlib import ExitStack
from itertools import chain, islice


def _take(gen, n):
    return islice(gen, n)

import numpy as np
import ml_dtypes

import concourse.mybir as mybir
import concourse.tile as tile
from concourse import bacc
from concourse import dve_ops
from concourse.bass_utils import run_bass_kernel_spmd
from concourse.dve_spec import Spec, Src0, C0, C1
from concourse.dve_ops import DveOp
from concourse.masks import make_identity

BF16 = mybir.dt.bfloat16
F32 = mybir.dt.float32
U16 = mybir.dt.uint16

B, S, D, H = 4, 2048, 1024, 16
HD = D // H          # 64
T = B * S            # 8192 tokens
NB = D // 128        # 8 d-tiles
SQC = 512            # query-chunk
NSQ = S // SQC       # 4 chunks per batch
NSK = S // 128       # 16 key tiles per batch
EXP = mybir.ActivationFunctionType.Exp
IDENT = mybir.ActivationFunctionType.Identity

# Schraudolph bf16 exp: bits = round(x*scale*128/ln2 + (127*128 - C)).
SCH_A = 0.125 * 128.0 / float(np.log(2.0))
SCH_B = 127.0 * 128.0 - 5.35

# which key-tile indices (of 16) run exp on the Vector engine
DVE_SKS = frozenset((1, 4, 7, 10, 13))

_CACHE = {}


def _register_exp_op():
    for op in dve_ops.OPS:
        if op.name == "ANT_EXP_SCH":
            return op
    op = DveOp(
        "ANT_EXP_SCH",
        Spec(
            body=Src0 * C0 + C1,
            reference=lambda in0, in1, s0, s1, imm2: (
                in0.astype(np.float32) * s0 + s1
            ),
        ),
        subdim=False,
        uops_sha={"v3": "2230da7084b02538"},
    )
    dve_ops.OPS.append(op)
    dve_ops._SUB_OPCODE_FOR_NAME[op.name] = (
        dve_ops._CUSTOM_DVE_ROW_BASE + len(dve_ops.OPS) - 1
    )
    dve_ops.CUSTOM_DVE_SPECS[op.name] = op.spec
    return op


EXP_OP = _register_exp_op()


def _build(reps=1):
    nc = bacc.Bacc("TRN2", target_bir_lowering=False, debug=False, num_devices=8)
    xt_d = nc.dram_tensor("xt", [D, T], BF16, kind="ExternalInput").ap()
    wq_d = nc.dram_tensor("wq", [D, 128], BF16, kind="ExternalInput").ap()
    wk_d = nc.dram_tensor("wk", [D, 128], BF16, kind="ExternalInput").ap()
    wv_d = nc.dram_tensor("wv", [D, 128], BF16, kind="ExternalInput").ap()
    bq_d = nc.dram_tensor("bq", [128, 1], F32, kind="ExternalInput").ap()
    bk_d = nc.dram_tensor("bk", [128, 1], F32, kind="ExternalInput").ap()
    bv_d = nc.dram_tensor("bv", [128, 1], F32, kind="ExternalInput").ap()
    wp_d = nc.dram_tensor("wp", [128, D], BF16, kind="ExternalInput").ap()
    yt_d = nc.dram_tensor("yt", [D, T], BF16, kind="ExternalOutput").ap()

    with tile.TileContext(nc) as tc, ExitStack() as ctx:
        const = ctx.enter_context(tc.tile_pool(name="const", bufs=1))
        xtp = ctx.enter_context(tc.tile_pool(name="xt", bufs=2))
        qkvp = ctx.enter_context(tc.tile_pool(name="qkv", bufs=3))
        vsp = ctx.enter_context(tc.tile_pool(name="vs", bufs=2))
        ep = ctx.enter_context(tc.tile_pool(name="e", bufs=6))
        otp = ctx.enter_context(tc.tile_pool(name="ot", bufs=2))
        tp = ctx.enter_context(tc.tile_pool(name="t", bufs=2))
        rp = ctx.enter_context(tc.tile_pool(name="r", bufs=2))
        r0p = ctx.enter_context(tc.tile_pool(name="r0", bufs=4))
        rbp = ctx.enter_context(tc.tile_pool(name="rb", bufs=4))
        yp = ctx.enter_context(tc.tile_pool(name="y", bufs=4))
        # PSUM: spair 2x2 banks + oA/oB 2 banks + scratch 2 banks = 8
        spp = ctx.enter_context(tc.tile_pool(name="sp", bufs=2, space="PSUM"))
        psp = ctx.enter_context(tc.tile_pool(name="ps", bufs=2, space="PSUM"))
        pso = ctx.enter_context(tc.tile_pool(name="pso", bufs=1, space="PSUM"))

        # persistent weights
        wq = const.tile([128, NB * 128], BF16)
        wk = const.tile([128, NB * 128], BF16)
        wv = const.tile([128, NB * 128], BF16)
        for w_sb, w_dr in ((wq, wq_d), (wk, wk_d), (wv, wv_d)):
            nc.sync.dma_start(
                w_sb[:].rearrange("p (n c) -> p n c", n=NB),
                w_dr.rearrange("(n p) c -> p n c", p=128))
        bq = const.tile([128, 1], F32)
        bk = const.tile([128, 1], F32)
        bv = const.tile([128, 1], F32)
        nc.sync.dma_start(bq[:], bq_d)
        nc.sync.dma_start(bk[:], bk_d)
        nc.sync.dma_start(bv[:], bv_d)
        wp = const.tile([128, D], BF16)
        nc.sync.dma_start(wp[:], wp_d)
        ident = const.tile([128, 128], BF16)
        make_identity(nc, ident[:])

        # per-batch state handed between pipeline stages
        st = {}

        def _qkv_chunk(b, w_sb, bias, dst, c):
            xt = st[b]["xt"]
            ps = psp.tile([128, SQC], F32, tag="ps")
            for d in range(NB):
                nc.tensor.matmul(
                    ps[:], w_sb[:, d * 128:(d + 1) * 128],
                    xt[:, d * S + c * SQC: d * S + (c + 1) * SQC],
                    start=(d == 0), stop=(d == NB - 1))
            nc.scalar.activation(
                dst[:, c * SQC:(c + 1) * SQC], ps[:], IDENT, bias=bias[:])

        def gen_pre_qk(b):
            """xt load (chunked) + q,k projections for batch b."""
            tok0 = b * S
            xt = xtp.tile([128, NB * S], BF16, tag="xt")
            st[b] = {"xt": xt}
            if b == 0:
                # chunk-major so the first token-chunk lands early
                for c in range(NSQ):
                    for d in range(NB):
                        nc.sync.dma_start(
                            xt[:, d * S + c * SQC: d * S + (c + 1) * SQC],
                            xt_d[d * 128:(d + 1) * 128,
                                 tok0 + c * SQC:tok0 + (c + 1) * SQC])
                    yield
            else:
                for d in range(NB):
                    nc.sync.dma_start(
                        xt[:, d * S:(d + 1) * S],
                        xt_d[d * 128:(d + 1) * 128, tok0:tok0 + S])
                    if d % 2:
                        yield
            qT = qkvp.tile([128, S], BF16, tag="qT")
            kT = qkvp.tile([128, S], BF16, tag="kT")
            st[b]["qT"] = qT
            st[b]["kT"] = kT
            for w_sb, bias, dst in ((wq, bq, qT), (wk, bk, kT)):
                for c in range(NSQ):
                    _qkv_chunk(b, w_sb, bias, dst, c)
                    yield

        def gen_pre_v(b):
            """v projection + transpose to natural layout for batch b."""
            vT = qkvp.tile([128, S], BF16, tag="vT")
            for c in range(NSQ):
                _qkv_chunk(b, wv, bv, vT, c)
                yield
            # v natural layout [vA(64) | 1 | vB(64) | 1] per key-tile
            v_sb = vsp.tile([128, NSK * 130], BF16, tag="vs")
            st[b]["v_sb"] = v_sb
            ones_ap = v_sb[:].rearrange("p (n c) -> p n c", c=65)[:, :, 64:65]
            nc.vector.memset(ones_ap, 1.0)
            yield
            for g in range(NSK // 4):
                pst = psp.tile([128, SQC], F32, tag="ps")
                pst_b = pst[:].bitcast(BF16)  # [128, 1024] bf16 view
                for j in range(4):
                    stk = 4 * g + j
                    nc.tensor.transpose(
                        pst_b[:, j * 128:(j + 1) * 128],
                        vT[:, stk * 128:(stk + 1) * 128], ident[:])
                dst = v_sb[:, g * 520:(g + 1) * 520].rearrange(
                    "p (n c) -> p n c", n=8, c=65)[:, :, 0:64]
                src = pst_b[:, 0:512].rearrange("p (n c) -> p n c", n=8)
                nc.vector.tensor_copy(dst, src)
                yield

        def gen_attn(b):
            """flash attention for batch b (yield per (chunk, sk) step).
            tsb0 holds denominator rows of chunks 0-1, tsb1 of chunks 2-3."""
            qT, kT = st[b]["qT"], st[b]["kT"]
            oT = otp.tile([128, S], BF16, tag="ot")
            tsb0 = tp.tile([97, SQC], F32, tag="t0")
            tsb1 = tp.tile([97, SQC], F32, tag="t1")
            nc.vector.memset(tsb0[:], 1.0)
            nc.vector.memset(tsb1[:], 1.0)
            st[b]["oT"] = oT
            st[b]["tsb"] = (tsb0, tsb1)
            for c in range(NSQ):
                q0 = c * SQC
                v_sb = st[b]["v_sb"]
                o_A = pso.tile([65, SQC], F32, tag="oA")
                o_B = pso.tile([65, SQC], F32, tag="oB")
                for sk in range(NSK):
                    k0 = sk * 128
                    sp = spp.tile([128, 2 * SQC], F32, tag="sp")
                    nc.tensor.matmul(sp[:, 0:SQC], kT[0:64, k0:k0 + 128],
                                     qT[0:64, q0:q0 + SQC],
                                     start=True, stop=True, tile_position=(0, 0))
                    nc.tensor.matmul(sp[:, SQC:2 * SQC], kT[64:128, k0:k0 + 128],
                                     qT[64:128, q0:q0 + SQC],
                                     start=True, stop=True, tile_position=(64, 0))
                    if sk in DVE_SKS:
                        e_u = ep.tile([128, 2 * SQC], U16, tag="ed")
                        nc.vector._custom_dve(EXP_OP, out=e_u[:], in0=sp[:],
                                              s0=SCH_A, s1=SCH_B)
                        e_ap = e_u[:].bitcast(BF16)
                    else:
                        e_s = ep.tile([128, 2 * SQC], BF16, tag="es")
                        nc.scalar.activation(e_s[:], sp[:], EXP, scale=0.125)
                        e_ap = e_s[:]
                    v0 = sk * 130
                    nc.tensor.matmul(o_A[:], v_sb[:, v0:v0 + 65],
                                     e_ap[:, 0:SQC],
                                     start=(sk == 0), stop=(sk == NSK - 1))
                    nc.tensor.matmul(o_B[:], v_sb[:, v0 + 65:v0 + 130],
                                     e_ap[:, SQC:2 * SQC],
                                     start=(sk == 0), stop=(sk == NSK - 1))
                    yield
                # stash unnormalized o (bf16) and the denominator row
                for h, o_ps in ((0, o_A), (1, o_B)):
                    row = (2 * (c % 2) + h) * 32
                    nc.vector.tensor_copy(
                        st[b]["tsb"][c // 2][row:row + 1, :], o_ps[64:65, :])
                    nc.vector.tensor_copy(oT[h * 64:(h + 1) * 64, q0:q0 + SQC],
                                          o_ps[0:64, :])
                yield

        def gen_post_half(b, half):
            """normalize + partial-proj + store for chunks 2h, 2h+1 of b."""
            tok0 = b * S
            oT = st[b]["oT"]
            tsb = st[b]["tsb"][half]
            rsb = rp.tile([97, SQC], F32, tag=f"r{half}t")
            nc.vector.reciprocal(rsb[:], tsb[:])
            yield
            for c in (2 * half, 2 * half + 1):
                q0 = c * SQC
                for h in range(2):
                    row = (2 * (c % 2) + h) * 32
                    if row == 0:
                        r_ap = rsb[0:1, :]
                    else:
                        r0 = r0p.tile([1, SQC], F32, tag="r0")
                        nc.gpsimd.tensor_copy(r0[:], rsb[row:row + 1, :])
                        r_ap = r0[:]
                    rb = rbp.tile([128, SQC], F32, tag="rb")
                    nc.gpsimd.partition_broadcast(rb[:], r_ap)
                    nc.vector.tensor_mul(
                        oT[h * 64:(h + 1) * 64, q0:q0 + SQC],
                        oT[h * 64:(h + 1) * 64, q0:q0 + SQC],
                        rb[h * 64:(h + 1) * 64, :])
                    yield
            # partial proj: yT[ct, tok] = wp[:, ct].T @ oT
            for ct in range(NB):
                y = yp.tile([128, 2 * SQC], BF16, tag="y")
                for ci, c in enumerate((2 * half, 2 * half + 1)):
                    ps = psp.tile([128, SQC], F32, tag="ps")
                    nc.tensor.matmul(ps[:], wp[:, ct * 128:(ct + 1) * 128],
                                     oT[:, c * SQC:(c + 1) * SQC],
                                     start=True, stop=True)
                    nc.vector.tensor_copy(
                        y[:, ci * SQC:(ci + 1) * SQC], ps[:])
                    yield
                nc.sync.dma_start(
                    yt_d[ct * 128:(ct + 1) * 128,
                         tok0 + 2 * half * SQC:tok0 + (2 * half + 2) * SQC],
                    y[:])

        def interleave(main, filler, n_main, n_fill):
            """emit main and filler streams at proportional rates."""
            ratio = max(n_fill, 1) / max(n_main, 1)
            credit = 0.0
            for mi in main:
                credit += ratio
                while credit >= 1.0:
                    credit -= 1.0
                    if next(filler, StopIteration) is StopIteration:
                        credit = -1e18
                        break
            for _ in filler:
                pass

        N_ATTN_HALF = 2 * (NSK + 1)
        N_PREQK = NSQ + 2 * NSQ
        N_PREV = NSQ + 1 + NSK // 4
        N_POST = 1 + 4 + 2 * NB

        for _ in range(reps):
            for _ in gen_pre_qk(0):
                pass
            interleave(gen_pre_v(0), gen_pre_qk(1), N_PREV, N_PREQK)
            for b in range(B):
                at = gen_attn(b)
                f1, n1 = [], 0
                if b - 1 >= 0:
                    f1.append(gen_post_half(b - 1, 1))
                    n1 += N_POST
                if b + 1 < B:
                    f1.append(gen_pre_v(b + 1))
                    n1 += N_PREV
                interleave(_take(at, N_ATTN_HALF), chain(*f1), N_ATTN_HALF, n1)
                f2, n2 = [gen_post_half(b, 0)], N_POST
                if b + 2 < B:
                    f2.append(gen_pre_qk(b + 2))
                    n2 += N_PREQK
                interleave(at, chain(*f2), N_ATTN_HALF, n2)
            for _ in gen_post_half(B - 1, 1):
                pass
    nc.compile()
    return nc


def _get_nc(reps=1):
    key = f"nc{reps}"
    if key not in _CACHE:
        _CACHE[key] = _build(reps)
    return _CACHE[key]


def make_in_maps(x, w_qkv, b_qkv, w_proj):
    """Host-side sharding: slice/cast per-core inputs."""
    bf16 = ml_dtypes.bfloat16
    xt = np.ascontiguousarray(
        np.asarray(x, dtype=np.float32).reshape(T, D).T).astype(bf16)
    w_qkv = np.asarray(w_qkv, dtype=np.float32)
    b_qkv = np.asarray(b_qkv, dtype=np.float32)
    w_proj = np.asarray(w_proj, dtype=np.float32)
    in_maps = []
    for p in range(8):
        c0 = p * 128          # first of the 128 head-pair columns
        in_maps.append({
            "xt": xt,
            "wq": np.ascontiguousarray(w_qkv[:, c0:c0 + 128]).astype(bf16),
            "wk": np.ascontiguousarray(w_qkv[:, D + c0:D + c0 + 128]).astype(bf16),
            "wv": np.ascontiguousarray(w_qkv[:, 2 * D + c0:2 * D + c0 + 128]).astype(bf16),
            "bq": b_qkv[c0:c0 + 128].reshape(128, 1).copy(),
            "bk": b_qkv[D + c0:D + c0 + 128].reshape(128, 1).copy(),
            "bv": b_qkv[2 * D + c0:2 * D + c0 + 128].reshape(128, 1).copy(),
            "wp": np.ascontiguousarray(w_proj[c0:c0 + 128, :]).astype(bf16),
        })
    return in_maps


def combine_outputs(results, b_proj):
    """Host-side unshard: sum partial yT, transpose back, add bias."""
    acc = np.zeros((D, T), np.float32)
    for r in results:
        acc += np.asarray(r["yt"], dtype=np.float32)
    y = acc.T.reshape(B, S, D) + np.asarray(b_proj, dtype=np.float32)
    return y.astype(np.float32)


def kernel(x, w_qkv, b_qkv, w_proj, b_proj):
    nc = _get_nc()
    in_maps = make_in_maps(x, w_qkv, b_qkv, w_proj)
    res = run_bass_kernel_spmd(nc, in_maps, list(range(8)))
    return combine_outputs(res.results, b_proj)
